# revision 1
# baseline (speedup 1.0000x reference)
"""Grid pooling (segment mean over rectangular grid cells) on 8 trn2 cores.

Math: row/col masks induce contiguous run-segments along H and W, so every
grid cell is a rectangle and the whole op factorizes per channel as

    out_c = A_h @ diag-scale( A_h^T @ X_c @ A_w ) @ A_w^T

with one-hot segment-assignment matrices A_h [H, NR], A_w [W, NC] built on
host from the tiny masks. Channels (64) are sharded 8-way across cores, so
each core runs 8 independent 768x768 channel planes through 4 matmul stages:

  1. R^T  = X_c^T  @ A_h      (row-segment sums;   lhsT = X chunks)
  2. S^T  = A_w^T  @ R^T      (col-segment sums;   lhsT = A_w chunks)
     S̄^T = S^T * 1/(n_r*n_q) (DVE multiply while copying PSUM->SBUF)
  3. U    = S̄     @ A_w^Tb   (broadcast cols back; lhsT = S̄^T chunks)
  4. OUT  = A_h b  @ U        (broadcast rows back; lhsT = A_h^T chunks)

All HBM<->SBUF transfers are fully contiguous: the host pre-permutes the
input to per-core planar [CL, 128, HK, W] tiles and un-permutes the output.
The device program is independent of mask contents (only the matrix *data*
changes), so it is built and compiled once per process.
"""

import numpy as np

from concourse import bacc, tile
import concourse.mybir as mybir
from concourse.bass_utils import run_bass_kernel_spmd

H = 768
W = 768
C = 64
NCORES = 8
CL = C // NCORES          # channels per core
HK = H // 128             # 6 H-chunks (contraction / output chunks)
WK = W // 128             # 6 W-chunks
NRP = 256                 # padded row-segment count (real ~192)
NCP = 256                 # padded col-segment count
NB = 384                  # free-dim tile for broadcast stages (768 = 2*384)

DT = mybir.dt.float32     # on-chip data dtype for X/R/S/U/out
F32 = mybir.dt.float32

_cached = {}


def _segment_ids(mask: np.ndarray) -> np.ndarray:
    """mask [L] binary -> segment ids via rising edges (pixel 0 -> seg 0)."""
    m = mask.astype(np.int64)
    prev = np.concatenate([[0], m[:-1]])
    rising = (m == 1) & (prev == 0)
    rising[0] = False
    return np.cumsum(rising.astype(np.int64)).astype(np.int32)


def _build_program():
    nc = bacc.Bacc("TRN2", target_bir_lowering=False, debug=False,
                   num_devices=NCORES)

    x_d = nc.dram_tensor("x", [CL, 128, HK, W], DT, kind="ExternalInput")
    ahn_d = nc.dram_tensor("ahn", [128, HK, NRP], DT, kind="ExternalInput")
    awn_d = nc.dram_tensor("awn", [128, WK, NCP], DT, kind="ExternalInput")
    inv_d = nc.dram_tensor("invt", [128, NCP // 128, NRP], F32,
                           kind="ExternalInput")
    awtb_d = nc.dram_tensor("awtb", [128, NCP // 128, W], DT,
                            kind="ExternalInput")
    ahtb_d = nc.dram_tensor("ahtb", [128, NRP // 128, H], DT,
                            kind="ExternalInput")
    o_d = nc.dram_tensor("o", [CL, HK, 128, W], DT, kind="ExternalOutput")

    with tile.TileContext(nc) as tc:
        with (
            tc.tile_pool(name="const", bufs=1) as constp,
            tc.tile_pool(name="xp", bufs=3) as xp,
            tc.tile_pool(name="rp", bufs=2) as rp,
            tc.tile_pool(name="sp", bufs=2) as sp,
            tc.tile_pool(name="up", bufs=2) as up,
            tc.tile_pool(name="op", bufs=4) as op_,
            tc.tile_pool(name="psr", bufs=2, space="PSUM") as psr,
            tc.tile_pool(name="pss", bufs=1, space="PSUM") as pss,
            tc.tile_pool(name="psu", bufs=2, space="PSUM") as psu,
            tc.tile_pool(name="pso", bufs=2, space="PSUM") as pso,
        ):
            ahn = constp.tile([128, HK, NRP], DT)
            nc.sync.dma_start(ahn[:], ahn_d[:])
            awn = constp.tile([128, WK, NCP], DT)
            nc.sync.dma_start(awn[:], awn_d[:])
            invt = constp.tile([128, NCP // 128, NRP], F32)
            nc.sync.dma_start(invt[:], inv_d[:])
            awtb = constp.tile([128, NCP // 128, W], DT)
            nc.sync.dma_start(awtb[:], awtb_d[:])
            ahtb = constp.tile([128, NRP // 128, H], DT)
            nc.sync.dma_start(ahtb[:], ahtb_d[:])

            for c in range(CL):
                xc = xp.tile([128, HK, W], DT)
                nc.sync.dma_start(xc[:], x_d[c])

                # stage 1: R^T[j, r] per W-chunk m (contract H in 6 chunks)
                rc = rp.tile([128, WK, NRP], DT)
                for m in range(WK):
                    pr = psr.tile([128, NRP], F32)
                    for k in range(HK):
                        nc.tensor.matmul(
                            pr[:],
                            xc[:, k, 128 * m:128 * m + 128],
                            ahn[:, k, :],
                            start=(k == 0), stop=(k == HK - 1),
                        )
                    nc.vector.tensor_copy(rc[:, m, :], pr[:])

                # stage 2: S^T[q, r] (contract W in 6 chunks), scale by
                # 1/(n_r*n_q) while copying out of PSUM
                sc = sp.tile([128, NCP // 128, NRP], DT)
                for mq in range(NCP // 128):
                    ps = pss.tile([128, NRP], F32)
                    for k in range(WK):
                        nc.tensor.matmul(
                            ps[:],
                            awn[:, k, 128 * mq:128 * mq + 128],
                            rc[:, k, :],
                            start=(k == 0), stop=(k == WK - 1),
                        )
                    nc.vector.tensor_mul(sc[:, mq, :], ps[:], invt[:, mq, :])

                # stage 3: U[r, j] = S̄[r, col_ids(j)] (contract q in 2 chunks)
                uc = up.tile([128, NRP // 128, W], DT)
                for mr in range(NRP // 128):
                    for n in range(W // NB):
                        pu = psu.tile([128, NB], F32)
                        for k in range(NCP // 128):
                            nc.tensor.matmul(
                                pu[:],
                                sc[:, k, 128 * mr:128 * mr + 128],
                                awtb[:, k, NB * n:NB * n + NB],
                                start=(k == 0), stop=(k == NCP // 128 - 1),
                            )
                        nc.vector.tensor_copy(uc[:, mr, NB * n:NB * n + NB],
                                              pu[:])

                # stage 4: OUT[i, j] = U[row_ids(i), j] (contract r in 2 chunks)
                for m in range(HK):
                    oc = op_.tile([128, W], DT)
                    for n in range(W // NB):
                        po = pso.tile([128, NB], F32)
                        for k in range(NRP // 128):
                            nc.tensor.matmul(
                                po[:],
                                ahtb[:, k, 128 * m:128 * m + 128],
                                uc[:, k, NB * n:NB * n + NB],
                                start=(k == 0), stop=(k == NRP // 128 - 1),
                            )
                        nc.vector.tensor_copy(oc[:, NB * n:NB * n + NB], po[:])
                    nc.sync.dma_start(o_d[c, m], oc[:])

    nc.compile()
    return nc


def _get_program():
    if "nc" not in _cached:
        _cached["nc"] = _build_program()
    return _cached["nc"]


def _np_dt():
    return np.float32 if DT == mybir.dt.float32 else np.dtype("bfloat16")


def _prepare(input, h_mask, v_mask):
    x = np.asarray(input, dtype=np.float32)
    hm = np.asarray(h_mask, dtype=np.int32)
    vm = np.asarray(v_mask, dtype=np.int32)
    assert x.shape == (1, H, W, C), x.shape

    row_ids = _segment_ids(hm[0])
    col_ids = _segment_ids(vm[0])
    nr = int(row_ids[-1]) + 1
    ncs = int(col_ids[-1]) + 1
    assert nr <= NRP and ncs <= NCP, (nr, ncs)

    n_r = np.bincount(row_ids, minlength=NRP).astype(np.float64)  # [NRP]
    n_q = np.bincount(col_ids, minlength=NCP).astype(np.float64)  # [NCP]

    npdt = np.float32  # host dtype for DT==fp32

    # one-hot assignment matrices
    ah = np.zeros((H, NRP), np.float32)
    ah[np.arange(H), row_ids] = 1.0
    aw = np.zeros((W, NCP), np.float32)
    aw[np.arange(W), col_ids] = 1.0

    inv = np.zeros((NCP, NRP), np.float64)
    valid = np.outer(n_q > 0, n_r > 0)
    denom = np.outer(n_q, n_r)
    inv[valid] = 1.0 / denom[valid]

    ahn_dev = np.ascontiguousarray(
        ah.reshape(HK, 128, NRP).transpose(1, 0, 2)).astype(npdt)
    awn_dev = np.ascontiguousarray(
        aw.reshape(WK, 128, NCP).transpose(1, 0, 2)).astype(npdt)
    inv_dev = np.ascontiguousarray(
        inv.reshape(NCP // 128, 128, NRP).transpose(1, 0, 2)).astype(np.float32)
    awtb_dev = np.ascontiguousarray(
        aw.T.reshape(NCP // 128, 128, W).transpose(1, 0, 2)).astype(npdt)
    ahtb_dev = np.ascontiguousarray(
        ah.T.reshape(NRP // 128, 128, H).transpose(1, 0, 2)).astype(npdt)

    # per-core planar input: [CL, 128(p), HK(h0), W] with h = 128*h0 + p
    x64 = x[0].transpose(2, 0, 1)  # [C, H, W]
    in_maps = []
    for core in range(NCORES):
        xc = x64[CL * core:CL * (core + 1)]  # [CL, H, W]
        xdev = np.ascontiguousarray(
            xc.reshape(CL, HK, 128, W).transpose(0, 2, 1, 3)).astype(npdt)
        in_maps.append({
            "x": xdev,
            "ahn": ahn_dev,
            "awn": awn_dev,
            "invt": inv_dev,
            "awtb": awtb_dev,
            "ahtb": ahtb_dev,
        })
    return in_maps


def _assemble(results):
    out = np.empty((1, H, W, C), np.float32)
    for core in range(NCORES):
        o = np.asarray(results[core]["o"], dtype=np.float32)  # [CL, HK, 128, W]
        oc = o.reshape(CL, H, W)                              # h = 128*m + p
        out[0, :, :, CL * core:CL * (core + 1)] = oc.transpose(1, 2, 0)
    return out


def run(inputs: dict, trace: bool = False, **kwargs):
    """Full pipeline; returns (output, BassKernelResults)."""
    nc = _get_program()
    in_maps = _prepare(**inputs)
    res = run_bass_kernel_spmd(nc, in_maps, list(range(NCORES)),
                               trace=trace, **kwargs)
    return _assemble(res.results), res


def kernel(**inputs) -> np.ndarray:
    out, _ = run(inputs, trace=False)
    return out



# revision 5
# speedup vs baseline: 2.7446x; 2.7446x over previous
"""Grid pooling (segment mean over rectangular grid cells) on 8 trn2 cores.

Math: row/col masks induce contiguous run-segments along H and W, so every
grid cell is a rectangle and the whole op factorizes per channel as

    out_c = A_h @ diag-scale( A_h^T @ X_c @ A_w ) @ A_w^T

with one-hot segment-assignment matrices A_h [H, NR], A_w [W, NC] built on
host from the tiny masks. Channels (64) are sharded 8-way across cores, so
each core runs 8 independent 768x768 channel planes through 4 matmul stages:

  1. R^T  = X_c^T  @ A_h      (row-segment sums;   lhsT = X chunks)
  2. S^T  = A_w^T  @ R^T      (col-segment sums;   lhsT = A_w chunks)
     S̄^T = S^T * 1/(n_r*n_q) (DVE multiply while copying PSUM->SBUF)
  3. U    = S̄     @ A_w^Tb   (broadcast cols back; lhsT = S̄^T chunks)
  4. OUT  = A_h b  @ U        (broadcast rows back; lhsT = A_h^T chunks)

Data is bf16 on-chip (fp32 PSUM accumulation, fp32 scale factors), which
runs the PE at 1 cycle/row instead of fp32's 4 and halves HBM traffic.
The segment-count padding NRP/NCP is chosen at runtime from the actual
masks (rounded up to a multiple of 64) and the program is cached per
(NRP, NCP) pair.

All HBM<->SBUF transfers are fully contiguous: the host pre-permutes the
input to per-core planar [CL, 128, HK, W] tiles and un-permutes the output.
"""

import numpy as np
import ml_dtypes

from concourse import bacc, tile
import concourse.mybir as mybir
from concourse.bass_utils import run_bass_kernel_spmd

H = 768
W = 768
C = 64
NCORES = 8
CL = C // NCORES          # channels per core
HK = H // 128             # 6 H-chunks (contraction / output chunks)
WK = W // 128             # 6 W-chunks
NB = 384                  # free-dim tile for broadcast stages (768 = 2*384)

DT = mybir.dt.bfloat16    # on-chip data dtype for X/R/S/U/out
F32 = mybir.dt.float32
NPDT = ml_dtypes.bfloat16

_cached = {}


def _segment_ids(mask: np.ndarray) -> np.ndarray:
    """mask [L] binary -> segment ids via rising edges (pixel 0 -> seg 0)."""
    m = mask.astype(np.int64)
    prev = np.concatenate([[0], m[:-1]])
    rising = (m == 1) & (prev == 0)
    rising[0] = False
    return np.cumsum(rising.astype(np.int64)).astype(np.int32)


def _blocks(n: int):
    """Split n into partition blocks of <=128: [(off, size), ...]."""
    return [(off, min(128, n - off)) for off in range(0, n, 128)]


def _build_program(NRP: int, NCP: int):
    QP = 128 * len(_blocks(NCP))   # partition-padded NCP
    RP = 128 * len(_blocks(NRP))   # partition-padded NRP
    qblocks = _blocks(NCP)
    rblocks = _blocks(NRP)

    nc = bacc.Bacc("TRN2", target_bir_lowering=False, debug=False,
                   num_devices=NCORES)

    x_d = nc.dram_tensor("x", [CL, 128, HK, W], DT, kind="ExternalInput")
    ahn_d = nc.dram_tensor("ahn", [128, HK, NRP], DT, kind="ExternalInput")
    awn_d = nc.dram_tensor("awn", [128, WK, NCP], DT, kind="ExternalInput")
    inv_d = nc.dram_tensor("invt", [128, QP // 128, NRP], F32,
                           kind="ExternalInput")
    awtb_d = nc.dram_tensor("awtb", [128, QP // 128, W], DT,
                            kind="ExternalInput")
    ahtb_d = nc.dram_tensor("ahtb", [128, RP // 128, H], DT,
                            kind="ExternalInput")
    o_d = nc.dram_tensor("o", [CL, HK, 128, W], DT, kind="ExternalOutput")

    with tile.TileContext(nc) as tc:
        with (
            tc.tile_pool(name="const", bufs=1) as constp,
            tc.tile_pool(name="xp", bufs=3) as xp,
            tc.tile_pool(name="rp", bufs=2) as rp,
            tc.tile_pool(name="sp", bufs=2) as sp,
            tc.tile_pool(name="up", bufs=2) as up,
            tc.tile_pool(name="op", bufs=4) as op_,
            tc.tile_pool(name="psr", bufs=2, space="PSUM") as psr,
            tc.tile_pool(name="pss", bufs=1, space="PSUM") as pss,
            tc.tile_pool(name="psu", bufs=2, space="PSUM") as psu,
            tc.tile_pool(name="pso", bufs=2, space="PSUM") as pso,
        ):
            ahn = constp.tile([128, HK, NRP], DT)
            nc.sync.dma_start(ahn[:], ahn_d[:])
            awn = constp.tile([128, WK, NCP], DT)
            nc.sync.dma_start(awn[:], awn_d[:])
            invt = constp.tile([128, QP // 128, NRP], F32)
            nc.sync.dma_start(invt[:], inv_d[:])
            awtb = constp.tile([128, QP // 128, W], DT)
            nc.sync.dma_start(awtb[:], awtb_d[:])
            ahtb = constp.tile([128, RP // 128, H], DT)
            nc.sync.dma_start(ahtb[:], ahtb_d[:])

            # only Vector (DVE) and Scalar (Act) engines can read PSUM on TRN2
            copy_engines = [nc.vector.tensor_copy, nc.scalar.copy]

            for c in range(CL):
                xc = xp.tile([128, HK, W], DT)
                nc.sync.dma_start(xc[:], x_d[c])

                # stage 1: R^T[j, r] per W-chunk m (contract H in 6 chunks)
                rc = rp.tile([128, WK, NRP], DT)
                for m in range(WK):
                    pr = psr.tile([128, NRP], F32)
                    for k in range(HK):
                        nc.tensor.matmul(
                            pr[:],
                            xc[:, k, 128 * m:128 * m + 128],
                            ahn[:, k, :],
                            start=(k == 0), stop=(k == HK - 1),
                        )
                    nc.scalar.copy(rc[:, m, :], pr[:])

                # stage 2: S^T[q, r] (contract W in 6 chunks), scale by
                # 1/(n_r*n_q) while copying out of PSUM
                sc = sp.tile([128, QP // 128, NRP], DT)
                for bi, (qo, qs) in enumerate(qblocks):
                    ps = pss.tile([128, NRP], F32)
                    for k in range(WK):
                        nc.tensor.matmul(
                            ps[0:qs, :],
                            awn[:, k, qo:qo + qs],
                            rc[:, k, :],
                            start=(k == 0), stop=(k == WK - 1),
                        )
                    nc.vector.tensor_mul(sc[0:qs, bi, :], ps[0:qs, :],
                                         invt[0:qs, bi, :])

                # stage 3: U[r, j] = S̄[r, col_ids(j)] (contract q blocks)
                uc = up.tile([128, RP // 128, W], DT)
                for ri, (ro, rs) in enumerate(rblocks):
                    for n in range(W // NB):
                        pu = psu.tile([128, NB], F32)
                        for bi, (qo, qs) in enumerate(qblocks):
                            nc.tensor.matmul(
                                pu[0:rs, :],
                                sc[0:qs, bi, ro:ro + rs],
                                awtb[0:qs, bi, NB * n:NB * n + NB],
                                start=(bi == 0), stop=(bi == len(qblocks) - 1),
                            )
                        nc.scalar.copy(uc[0:rs, ri, NB * n:NB * n + NB],
                                       pu[0:rs, :])

                # stage 4: OUT[i, j] = U[row_ids(i), j] (contract r blocks)
                for m in range(HK):
                    oc = op_.tile([128, W], DT)
                    for n in range(W // NB):
                        po = pso.tile([128, NB], F32)
                        for ri, (ro, rs) in enumerate(rblocks):
                            nc.tensor.matmul(
                                po[:],
                                ahtb[0:rs, ri, 128 * m:128 * m + 128],
                                uc[0:rs, ri, NB * n:NB * n + NB],
                                start=(ri == 0), stop=(ri == len(rblocks) - 1),
                            )
                        copy_engines[(m * (W // NB) + n) % 2](
                            oc[:, NB * n:NB * n + NB], po[:])
                    nc.sync.dma_start(o_d[c, m], oc[:])

    nc.compile()
    return nc


def _get_program(NRP: int, NCP: int):
    key = (NRP, NCP)
    if key not in _cached:
        _cached[key] = _build_program(NRP, NCP)
    return _cached[key]


def _prepare(input, h_mask, v_mask):
    x = np.asarray(input, dtype=np.float32)
    hm = np.asarray(h_mask, dtype=np.int32)
    vm = np.asarray(v_mask, dtype=np.int32)
    assert x.shape == (1, H, W, C), x.shape

    row_ids = _segment_ids(hm[0])
    col_ids = _segment_ids(vm[0])
    nr = int(row_ids[-1]) + 1
    ncs = int(col_ids[-1]) + 1
    NRP = ((nr + 63) // 64) * 64
    NCP = ((ncs + 63) // 64) * 64
    QP = 128 * len(_blocks(NCP))
    RP = 128 * len(_blocks(NRP))

    n_r = np.bincount(row_ids, minlength=NRP).astype(np.float64)  # [NRP]
    n_q = np.bincount(col_ids, minlength=NCP).astype(np.float64)  # [NCP]

    # one-hot assignment matrices
    ah = np.zeros((H, NRP), np.float32)
    ah[np.arange(H), row_ids] = 1.0
    aw = np.zeros((W, NCP), np.float32)
    aw[np.arange(W), col_ids] = 1.0

    inv = np.zeros((QP, NRP), np.float64)
    valid = np.outer(n_q > 0, n_r > 0)
    denom = np.outer(n_q, n_r)
    inv[:NCP][valid] = 1.0 / denom[valid]

    awt = np.zeros((QP, W), np.float32)
    awt[:NCP] = aw.T
    aht = np.zeros((RP, H), np.float32)
    aht[:NRP] = ah.T

    ahn_dev = np.ascontiguousarray(
        ah.reshape(HK, 128, NRP).transpose(1, 0, 2)).astype(NPDT)
    awn_dev = np.ascontiguousarray(
        aw.reshape(WK, 128, NCP).transpose(1, 0, 2)).astype(NPDT)
    inv_dev = np.ascontiguousarray(
        inv.reshape(QP // 128, 128, NRP).transpose(1, 0, 2)).astype(np.float32)
    awtb_dev = np.ascontiguousarray(
        awt.reshape(QP // 128, 128, W).transpose(1, 0, 2)).astype(NPDT)
    ahtb_dev = np.ascontiguousarray(
        aht.reshape(RP // 128, 128, H).transpose(1, 0, 2)).astype(NPDT)

    # per-core planar input: [CL, 128(p), HK(h0), W] with h = 128*h0 + p
    x64 = x[0].transpose(2, 0, 1)  # [C, H, W]
    in_maps = []
    for core in range(NCORES):
        xc = x64[CL * core:CL * (core + 1)]  # [CL, H, W]
        xdev = np.ascontiguousarray(
            xc.reshape(CL, HK, 128, W).transpose(0, 2, 1, 3)).astype(NPDT)
        in_maps.append({
            "x": xdev,
            "ahn": ahn_dev,
            "awn": awn_dev,
            "invt": inv_dev,
            "awtb": awtb_dev,
            "ahtb": ahtb_dev,
        })
    return in_maps, NRP, NCP


def _assemble(results):
    out = np.empty((1, H, W, C), np.float32)
    for core in range(NCORES):
        o = np.asarray(results[core]["o"]).astype(np.float32)  # [CL,HK,128,W]
        oc = o.reshape(CL, H, W)                               # h = 128*m + p
        out[0, :, :, CL * core:CL * (core + 1)] = oc.transpose(1, 2, 0)
    return out


def run(inputs: dict, trace: bool = False, **kwargs):
    """Full pipeline; returns (output, BassKernelResults)."""
    in_maps, NRP, NCP = _prepare(**inputs)
    nc = _get_program(NRP, NCP)
    res = run_bass_kernel_spmd(nc, in_maps, list(range(NCORES)),
                               trace=trace, **kwargs)
    return _assemble(res.results), res


def kernel(**inputs) -> np.ndarray:
    out, _ = run(inputs, trace=False)
    return out


# revision 9
# speedup vs baseline: 3.4673x; 1.2633x over previous
"""Grid pooling (segment mean over rectangular grid cells) on 8 trn2 cores.

Math: row/col masks induce contiguous run-segments along H and W, so every
grid cell is a rectangle and the whole op factorizes per channel as

    out_c = A_h @ diag-scale( A_h^T @ X_c @ A_w ) @ A_w^T

with one-hot segment-assignment matrices A_h [H, NR], A_w [W, NC] built on
host from the tiny masks. Channels (64) are sharded 8-way across cores, so
each core runs 8 independent 768x768 channel planes through 4 matmul stages:

  1. R^T  = X_c^T  @ A_h      (row-segment sums;   lhsT = X chunks)
  2. S^T  = A_w^T  @ R^T      (col-segment sums;   lhsT = A_w chunks)
     S̄^T = S^T * 1/(n_r*n_q) (DVE multiply while copying PSUM->SBUF)
  3. U    = S̄     @ A_w^Tb   (broadcast cols back; lhsT = S̄^T chunks)
  4. OUT  = A_h b  @ U        (broadcast rows back; lhsT = A_h^T chunks)

Data is bf16 on-chip (fp32 PSUM accumulation, fp32 scale factors): the PE
runs 1 cycle/row instead of fp32's 4 and HBM traffic halves.

Because segments are contiguous runs, the segments touched by one 128-row
output block (or one 384-column tile) span a narrow index range (~33 for
random masks). The r/q axes are therefore partitioned into mask-adapted,
possibly overlapping blocks of <=128 segments such that every output tile's
range lives in ONE block, making stages 3 and 4 single-k matmuls. Falls
back to fixed disjoint 128-blocks (multi-k accumulation) if a range is too
wide. The program is cached per blocking structure.
"""

import numpy as np
import ml_dtypes

from concourse import bacc, tile
import concourse.mybir as mybir
from concourse.bass_utils import run_bass_kernel_spmd

H = 768
W = 768
C = 64
NCORES = 8
CL = C // NCORES          # channels per core
HK = H // 128             # 6 H-chunks (contraction / output chunks)
WK = W // 128             # 6 W-chunks
NB = 384                  # free-dim tile for broadcast stages (768 = 2*384)

DT = mybir.dt.bfloat16    # on-chip data dtype for X/R/S/U/out
F32 = mybir.dt.float32
NPDT = ml_dtypes.bfloat16

_cached = {}


def _segment_ids(mask: np.ndarray) -> np.ndarray:
    """mask [L] binary -> segment ids via rising edges (pixel 0 -> seg 0)."""
    m = mask.astype(np.int64)
    prev = np.concatenate([[0], m[:-1]])
    rising = (m == 1) & (prev == 0)
    rising[0] = False
    return np.cumsum(rising.astype(np.int64)).astype(np.int32)


def _adapt_blocks(ids: np.ndarray, tile_len: int, nseg: int):
    """Partition the segment axis into blocks of <=128 ids such that the id
    range of every output tile of `tile_len` positions lies in one block.

    Returns (blocks, assign): blocks = [(start, width), ...] (may overlap by
    a shared boundary segment), assign[t] = [(block_idx, k_first), ...] the
    block(s) tile t accumulates over. Single-element lists on the fast path;
    falls back to fixed disjoint 128-blocks (multi-k) if a tile's range
    exceeds 128 segments.
    """
    L = len(ids)
    ntiles = L // tile_len
    ranges = [(int(ids[t * tile_len]), int(ids[(t + 1) * tile_len - 1]))
              for t in range(ntiles)]
    if all(hi - lo + 1 <= 128 for lo, hi in ranges):
        blocks, assign = [], []
        cur_start, cur_end = 0, -1          # current block [cur_start, cur_end]
        for lo, hi in ranges:
            if hi - cur_start + 1 <= 128 and cur_end >= 0:
                cur_end = max(cur_end, hi)
            else:
                if cur_end >= 0:
                    blocks.append((cur_start, cur_end - cur_start + 1))
                cur_start, cur_end = lo, hi
            assign.append(len(blocks))      # block this tile will land in
        blocks.append((cur_start, cur_end - cur_start + 1))
        return blocks, [[a] for a in assign]
    # fallback: fixed disjoint blocks, tiles accumulate over all their blocks
    blocks = [(off, min(128, nseg - off)) for off in range(0, nseg, 128)]
    assign = []
    for lo, hi in ranges:
        assign.append([b for b, (s, w) in enumerate(blocks)
                       if not (hi < s or lo > s + w - 1)])
    return blocks, assign


def _build_program(key):
    (NRP, NCP, rblocks, rassign, qblocks, qassign) = key
    RBn = len(rblocks)
    QBn = len(qblocks)

    nc = bacc.Bacc("TRN2", target_bir_lowering=False, debug=False,
                   num_devices=NCORES)

    x_d = nc.dram_tensor("x", [CL, 128, HK, W], DT, kind="ExternalInput")
    ahn_d = nc.dram_tensor("ahn", [128, HK, NRP], DT, kind="ExternalInput")
    awn_d = nc.dram_tensor("awn", [128, WK, NCP], DT, kind="ExternalInput")
    inv_d = nc.dram_tensor("invt", [128, QBn, NRP], F32, kind="ExternalInput")
    awtb_d = nc.dram_tensor("awtb", [128, QBn, W], DT, kind="ExternalInput")
    ahtb_d = nc.dram_tensor("ahtb", [128, RBn, H], DT, kind="ExternalInput")
    o_d = nc.dram_tensor("o", [CL, 128, HK, W], DT, kind="ExternalOutput")

    with tile.TileContext(nc) as tc:
        with (
            tc.tile_pool(name="const", bufs=1) as constp,
            tc.tile_pool(name="xp", bufs=3) as xp,
            tc.tile_pool(name="rp", bufs=2) as rp,
            tc.tile_pool(name="sp", bufs=2) as sp,
            tc.tile_pool(name="up", bufs=2) as up,
            tc.tile_pool(name="op", bufs=2) as op_,
            # 8 PSUM banks total: psr 2 + pss 1 + psu 2 + pso 3
            tc.tile_pool(name="psr", bufs=2, space="PSUM") as psr,
            tc.tile_pool(name="pss", bufs=1, space="PSUM") as pss,
            tc.tile_pool(name="psu", bufs=2, space="PSUM") as psu,
            tc.tile_pool(name="pso", bufs=3, space="PSUM") as pso,
        ):
            # two stage-1 results fit one 2KB PSUM bank when NRP <= 256
            pair1 = 2 * NRP * 4 <= 2048
            ahn = constp.tile([128, HK, NRP], DT)
            nc.sync.dma_start(ahn[:], ahn_d[:])
            awn = constp.tile([128, WK, NCP], DT)
            nc.sync.dma_start(awn[:], awn_d[:])
            invt = constp.tile([128, QBn, NRP], F32)
            nc.sync.dma_start(invt[:], inv_d[:])
            awtb = constp.tile([128, QBn, W], DT)
            nc.sync.dma_start(awtb[:], awtb_d[:])
            ahtb = constp.tile([128, RBn, H], DT)
            nc.sync.dma_start(ahtb[:], ahtb_d[:])

            for c in range(CL):
                xc = xp.tile([128, HK, W], DT)
                nc.sync.dma_start(xc[:], x_d[c])

                # stage 1: R^T[j, r] per W-chunk m (contract H in 6 chunks);
                # two m per PSUM bank, one paired copy out
                rc = rp.tile([128, WK, NRP], DT)
                if pair1:
                    for mp in range(WK // 2):
                        pr = psr.tile([128, 2, NRP], F32)
                        for half in range(2):
                            m = 2 * mp + half
                            for k in range(HK):
                                nc.tensor.matmul(
                                    pr[:, half, :],
                                    xc[:, k, 128 * m:128 * m + 128],
                                    ahn[:, k, :],
                                    start=(k == 0), stop=(k == HK - 1),
                                )
                        nc.scalar.copy(rc[:, 2 * mp:2 * mp + 2, :], pr[:])
                else:
                    for m in range(WK):
                        pr = psr.tile([128, NRP], F32)
                        for k in range(HK):
                            nc.tensor.matmul(
                                pr[:],
                                xc[:, k, 128 * m:128 * m + 128],
                                ahn[:, k, :],
                                start=(k == 0), stop=(k == HK - 1),
                            )
                        nc.scalar.copy(rc[:, m, :], pr[:])

                # stage 2: S^T[q, r] (contract W in 6 chunks), scale by
                # 1/(n_r*n_q) while copying out of PSUM
                sc = sp.tile([128, QBn, NRP], DT)
                for b, (qo, qs) in enumerate(qblocks):
                    ps = pss.tile([128, NRP], F32)
                    for k in range(WK):
                        nc.tensor.matmul(
                            ps[0:qs, :],
                            awn[:, k, qo:qo + qs],
                            rc[:, k, :],
                            start=(k == 0), stop=(k == WK - 1),
                        )
                    nc.vector.tensor_mul(sc[0:qs, b, :], ps[0:qs, :],
                                         invt[0:qs, b, :])

                # stage 3: U[r, j] = S̄[r, col_ids(j)] per adapted r-block
                uc = up.tile([128, RBn, W], DT)
                for ri, (ro, rs) in enumerate(rblocks):
                    for n in range(W // NB):
                        pu = psu.tile([128, NB], F32)
                        ks = qassign[n]
                        for j, b in enumerate(ks):
                            qo, qs = qblocks[b]
                            nc.tensor.matmul(
                                pu[0:rs, :],
                                sc[0:qs, b, ro:ro + rs],
                                awtb[0:qs, b, NB * n:NB * n + NB],
                                start=(j == 0), stop=(j == len(ks) - 1),
                            )
                        nc.scalar.copy(uc[0:rs, ri, NB * n:NB * n + NB],
                                       pu[0:rs, :])

                # stage 4: OUT[i, j] = U[row_ids(i), j] into the channel tile
                ocC = op_.tile([128, HK, W], DT)
                for m in range(HK):
                    for n in range(W // NB):
                        po = pso.tile([128, NB], F32)
                        ks = rassign[m]
                        for j, b in enumerate(ks):
                            ro, rs = rblocks[b]
                            nc.tensor.matmul(
                                po[:],
                                ahtb[0:rs, b, 128 * m:128 * m + 128],
                                uc[0:rs, b, NB * n:NB * n + NB],
                                start=(j == 0), stop=(j == len(ks) - 1),
                            )
                        nc.vector.tensor_copy(ocC[:, m, NB * n:NB * n + NB],
                                              po[:])
                nc.sync.dma_start(o_d[c], ocC[:])

    nc.compile()
    return nc


def _get_program(key):
    if key not in _cached:
        _cached[key] = _build_program(key)
    return _cached[key]


def _prepare(input, h_mask, v_mask):
    x = np.asarray(input, dtype=np.float32)
    hm = np.asarray(h_mask, dtype=np.int32)
    vm = np.asarray(v_mask, dtype=np.int32)
    assert x.shape == (1, H, W, C), x.shape

    row_ids = _segment_ids(hm[0])
    col_ids = _segment_ids(vm[0])
    nr = int(row_ids[-1]) + 1
    ncs = int(col_ids[-1]) + 1
    NRP = ((nr + 63) // 64) * 64
    NCP = ((ncs + 63) // 64) * 64

    rblocks, rassign = _adapt_blocks(row_ids, 128, nr)
    qblocks, qassign = _adapt_blocks(col_ids, NB, ncs)
    key = (NRP, NCP,
           tuple(rblocks), tuple(tuple(a) for a in rassign),
           tuple(qblocks), tuple(tuple(a) for a in qassign))

    n_r = np.bincount(row_ids, minlength=NRP).astype(np.float64)  # [NRP]
    n_q = np.bincount(col_ids, minlength=NCP).astype(np.float64)  # [NCP]

    # one-hot assignment matrices
    ah = np.zeros((H, NRP), np.float32)
    ah[np.arange(H), row_ids] = 1.0
    aw = np.zeros((W, NCP), np.float32)
    aw[np.arange(W), col_ids] = 1.0

    inv_full = np.zeros((NCP, NRP), np.float64)
    valid = np.outer(n_q > 0, n_r > 0)
    denom = np.outer(n_q, n_r)
    inv_full[valid] = 1.0 / denom[valid]

    # per-adapted-block partition layouts (zero padded to 128 partitions)
    QBn, RBn = len(qblocks), len(rblocks)
    inv_dev = np.zeros((128, QBn, NRP), np.float32)
    awtb_dev = np.zeros((128, QBn, W), np.float32)
    for b, (qo, qs) in enumerate(qblocks):
        inv_dev[0:qs, b, :] = inv_full[qo:qo + qs]
        awtb_dev[0:qs, b, :] = aw.T[qo:qo + qs]
    ahtb_dev = np.zeros((128, RBn, H), np.float32)
    for b, (ro, rs) in enumerate(rblocks):
        ahtb_dev[0:rs, b, :] = ah.T[ro:ro + rs]

    ahn_dev = np.ascontiguousarray(
        ah.reshape(HK, 128, NRP).transpose(1, 0, 2)).astype(NPDT)
    awn_dev = np.ascontiguousarray(
        aw.reshape(WK, 128, NCP).transpose(1, 0, 2)).astype(NPDT)
    inv_dev = np.ascontiguousarray(inv_dev)
    awtb_dev = awtb_dev.astype(NPDT)
    ahtb_dev = ahtb_dev.astype(NPDT)

    # per-core planar input: [CL, 128(p), HK(h0), W] with h = 128*h0 + p
    x64 = x[0].transpose(2, 0, 1)  # [C, H, W]
    in_maps = []
    for core in range(NCORES):
        xc = x64[CL * core:CL * (core + 1)]  # [CL, H, W]
        xdev = np.ascontiguousarray(
            xc.reshape(CL, HK, 128, W).transpose(0, 2, 1, 3)).astype(NPDT)
        in_maps.append({
            "x": xdev,
            "ahn": ahn_dev,
            "awn": awn_dev,
            "invt": inv_dev,
            "awtb": awtb_dev,
            "ahtb": ahtb_dev,
        })
    return in_maps, key


def _assemble(results):
    out = np.empty((1, H, W, C), np.float32)
    for core in range(NCORES):
        o = np.asarray(results[core]["o"]).astype(np.float32)  # [CL,128,HK,W]
        oc = o.transpose(0, 2, 1, 3).reshape(CL, H, W)         # h = 128*m + p
        out[0, :, :, CL * core:CL * (core + 1)] = oc.transpose(1, 2, 0)
    return out


def run(inputs: dict, trace: bool = False, **kwargs):
    """Full pipeline; returns (output, BassKernelResults)."""
    in_maps, key = _prepare(**inputs)
    nc = _get_program(key)
    res = run_bass_kernel_spmd(nc, in_maps, list(range(NCORES)),
                               trace=trace, **kwargs)
    return _assemble(res.results), res


def kernel(**inputs) -> np.ndarray:
    out, _ = run(inputs, trace=False)
    return out


# revision 11
# speedup vs baseline: 3.9300x; 1.1334x over previous
"""Grid pooling (segment mean over rectangular grid cells) on 8 trn2 cores.

Math: row/col masks induce contiguous run-segments along H and W, so every
grid cell is a rectangle and the whole op factorizes per channel as

    out_c = A_h @ diag-scale( A_h^T @ X_c @ A_w ) @ A_w^T

with one-hot segment-assignment matrices A_h [H, NR], A_w [W, NC] built on
host from the tiny masks. Channels (64) are sharded 8-way across cores, so
each core runs 8 independent 768x768 channel planes through 4 matmul stages:

  1. R^T  = X_c^T  @ A_h      (row-segment sums;   lhsT = X chunks)
  2. S^T  = A_w^T  @ R^T      (col-segment sums;   lhsT = A_w chunks)
     S̄^T = S^T * 1/(n_r*n_q) (DVE multiply while copying PSUM->SBUF)
  3. U    = S̄     @ A_w^Tb   (broadcast cols back; lhsT = S̄^T chunks)
  4. OUT  = A_h b  @ U        (broadcast rows back; lhsT = A_h^T chunks)

Data is bf16 on-chip (fp32 PSUM accumulation, fp32 scale factors): the PE
runs 1 cycle/row instead of fp32's 4 and HBM traffic halves.

Because segments are contiguous runs, the segments touched by one 128-row
output block (or one 384-column tile) span a narrow index range (~33 for
random masks). The r/q axes are therefore partitioned into mask-adapted,
possibly overlapping blocks of <=128 segments such that every output tile's
range lives in ONE block, making stages 3 and 4 single-k matmuls. Falls
back to fixed disjoint 128-blocks (multi-k accumulation) if a range is too
wide. The program is cached per blocking structure.
"""

import numpy as np
import ml_dtypes

from concourse import bacc, tile
import concourse.mybir as mybir
from concourse.bass_utils import run_bass_kernel_spmd

H = 768
W = 768
C = 64
NCORES = 8
CL = C // NCORES          # channels per core
HK = H // 128             # 6 H-chunks (contraction / output chunks)
WK = W // 128             # 6 W-chunks
NB = 384                  # free-dim tile for broadcast stages (768 = 2*384)

DT = mybir.dt.bfloat16    # on-chip data dtype for X/R/S/U/out
F32 = mybir.dt.float32
NPDT = ml_dtypes.bfloat16

_cached = {}


def _segment_ids(mask: np.ndarray) -> np.ndarray:
    """mask [L] binary -> segment ids via rising edges (pixel 0 -> seg 0)."""
    m = mask.astype(np.int64)
    prev = np.concatenate([[0], m[:-1]])
    rising = (m == 1) & (prev == 0)
    rising[0] = False
    return np.cumsum(rising.astype(np.int64)).astype(np.int32)


def _adapt_blocks(ids: np.ndarray, tile_len: int, nseg: int):
    """Partition the segment axis into blocks of <=128 ids such that the id
    range of every output tile of `tile_len` positions lies in one block.

    Returns (blocks, assign): blocks = [(start, width), ...] (may overlap by
    a shared boundary segment), assign[t] = [(block_idx, k_first), ...] the
    block(s) tile t accumulates over. Single-element lists on the fast path;
    falls back to fixed disjoint 128-blocks (multi-k) if a tile's range
    exceeds 128 segments.
    """
    L = len(ids)
    ntiles = L // tile_len
    ranges = [(int(ids[t * tile_len]), int(ids[(t + 1) * tile_len - 1]))
              for t in range(ntiles)]
    if all(hi - lo + 1 <= 128 for lo, hi in ranges):
        blocks, assign = [], []
        cur_start, cur_end = 0, -1          # current block [cur_start, cur_end]
        for lo, hi in ranges:
            if hi - cur_start + 1 <= 128 and cur_end >= 0:
                cur_end = max(cur_end, hi)
            else:
                if cur_end >= 0:
                    blocks.append((cur_start, cur_end - cur_start + 1))
                cur_start, cur_end = lo, hi
            assign.append(len(blocks))      # block this tile will land in
        blocks.append((cur_start, cur_end - cur_start + 1))
        return blocks, [[a] for a in assign]
    # fallback: fixed disjoint blocks, tiles accumulate over all their blocks
    blocks = [(off, min(128, nseg - off)) for off in range(0, nseg, 128)]
    assign = []
    for lo, hi in ranges:
        assign.append([b for b, (s, w) in enumerate(blocks)
                       if not (hi < s or lo > s + w - 1)])
    return blocks, assign


def _build_program(key):
    (NRP, NCP, rblocks, rassign, qblocks, qassign) = key
    RBn = len(rblocks)
    QBn = len(qblocks)

    nc = bacc.Bacc("TRN2", target_bir_lowering=False, debug=False,
                   num_devices=NCORES)

    x_d = nc.dram_tensor("x", [CL, 128, HK, W], DT, kind="ExternalInput")
    ahn_d = nc.dram_tensor("ahn", [128, HK, NRP], DT, kind="ExternalInput")
    awn_d = nc.dram_tensor("awn", [128, WK, NCP], DT, kind="ExternalInput")
    inv_d = nc.dram_tensor("invt", [128, QBn, NRP], F32, kind="ExternalInput")
    awtb_d = nc.dram_tensor("awtb", [128, QBn, W], DT, kind="ExternalInput")
    ahtb_d = nc.dram_tensor("ahtb", [128, RBn, H], DT, kind="ExternalInput")
    o_d = nc.dram_tensor("o", [CL, 128, HK, W], DT, kind="ExternalOutput")

    with tile.TileContext(nc) as tc:
        with (
            tc.tile_pool(name="const", bufs=1) as constp,
            tc.tile_pool(name="xp", bufs=3) as xp,
            tc.tile_pool(name="rp", bufs=2) as rp,
            tc.tile_pool(name="sp", bufs=2) as sp,
            tc.tile_pool(name="up", bufs=2) as up,
            tc.tile_pool(name="op", bufs=2) as op_,
            # 8 PSUM banks total: psr 2 + pss 1 + psu 2 + pso 3
            tc.tile_pool(name="psr", bufs=2, space="PSUM") as psr,
            tc.tile_pool(name="pss", bufs=1, space="PSUM") as pss,
            tc.tile_pool(name="psu", bufs=2, space="PSUM") as psu,
            tc.tile_pool(name="pso", bufs=3, space="PSUM") as pso,
        ):
            # two stage-1 results fit one 2KB PSUM bank when NRP <= 256
            pair1 = 2 * NRP * 4 <= 2048
            # first channel's input before the big broadcast constants so the
            # PE can start as early as possible (ahn is all stage 1 needs)
            xc0 = xp.tile([128, HK, W], DT)
            nc.sync.dma_start(xc0[:], x_d[0])
            ahn = constp.tile([128, HK, NRP], DT)
            nc.sync.dma_start(ahn[:], ahn_d[:])
            awn = constp.tile([128, WK, NCP], DT)
            nc.sync.dma_start(awn[:], awn_d[:])
            invt = constp.tile([128, QBn, NRP], F32)
            nc.sync.dma_start(invt[:], inv_d[:])
            awtb = constp.tile([128, QBn, W], DT)
            nc.sync.dma_start(awtb[:], awtb_d[:])
            ahtb = constp.tile([128, RBn, H], DT)
            nc.sync.dma_start(ahtb[:], ahtb_d[:])

            for c in range(CL):
                if c == 0:
                    xc = xc0
                else:
                    xc = xp.tile([128, HK, W], DT)
                    nc.sync.dma_start(xc[:], x_d[c])

                # stage 1: R^T[j, r] per W-chunk m (contract H in 6 chunks);
                # two m per PSUM bank, one paired copy out
                rc = rp.tile([128, WK, NRP], DT)
                if pair1:
                    for mp in range(WK // 2):
                        pr = psr.tile([128, 2, NRP], F32)
                        for half in range(2):
                            m = 2 * mp + half
                            for k in range(HK):
                                nc.tensor.matmul(
                                    pr[:, half, :],
                                    xc[:, k, 128 * m:128 * m + 128],
                                    ahn[:, k, :],
                                    start=(k == 0), stop=(k == HK - 1),
                                )
                        nc.scalar.copy(rc[:, 2 * mp:2 * mp + 2, :], pr[:])
                else:
                    for m in range(WK):
                        pr = psr.tile([128, NRP], F32)
                        for k in range(HK):
                            nc.tensor.matmul(
                                pr[:],
                                xc[:, k, 128 * m:128 * m + 128],
                                ahn[:, k, :],
                                start=(k == 0), stop=(k == HK - 1),
                            )
                        nc.scalar.copy(rc[:, m, :], pr[:])

                # stage 2: S^T[q, r] (contract W in 6 chunks), scale by
                # 1/(n_r*n_q) while copying out of PSUM
                sc = sp.tile([128, QBn, NRP], DT)
                for b, (qo, qs) in enumerate(qblocks):
                    ps = pss.tile([128, NRP], F32)
                    for k in range(WK):
                        nc.tensor.matmul(
                            ps[0:qs, :],
                            awn[:, k, qo:qo + qs],
                            rc[:, k, :],
                            start=(k == 0), stop=(k == WK - 1),
                        )
                    nc.vector.tensor_mul(sc[0:qs, b, :], ps[0:qs, :],
                                         invt[0:qs, b, :])

                # stage 3: U[r, j] = S̄[r, col_ids(j)] per adapted r-block
                uc = up.tile([128, RBn, W], DT)
                for ri, (ro, rs) in enumerate(rblocks):
                    for n in range(W // NB):
                        pu = psu.tile([128, NB], F32)
                        ks = qassign[n]
                        for j, b in enumerate(ks):
                            qo, qs = qblocks[b]
                            nc.tensor.matmul(
                                pu[0:rs, :],
                                sc[0:qs, b, ro:ro + rs],
                                awtb[0:qs, b, NB * n:NB * n + NB],
                                start=(j == 0), stop=(j == len(ks) - 1),
                            )
                        nc.scalar.copy(uc[0:rs, ri, NB * n:NB * n + NB],
                                       pu[0:rs, :])

                # stage 4: OUT[i, j] = U[row_ids(i), j] into the channel tile;
                # casts split 3:1 vector:scalar, output DMA'd in two halves
                ocC = op_.tile([128, HK, W], DT)
                for m in range(HK):
                    for n in range(W // NB):
                        po = pso.tile([128, NB], F32)
                        ks = rassign[m]
                        for j, b in enumerate(ks):
                            ro, rs = rblocks[b]
                            nc.tensor.matmul(
                                po[:],
                                ahtb[0:rs, b, 128 * m:128 * m + 128],
                                uc[0:rs, b, NB * n:NB * n + NB],
                                start=(j == 0), stop=(j == len(ks) - 1),
                            )
                        dst = ocC[:, m, NB * n:NB * n + NB]
                        if (2 * m + n) % 4 == 3:
                            nc.scalar.copy(dst, po[:])
                        else:
                            nc.vector.tensor_copy(dst, po[:])
                    if m == HK // 2 - 1:
                        nc.sync.dma_start(o_d[c][:, 0:HK // 2, :],
                                          ocC[:, 0:HK // 2, :])
                nc.sync.dma_start(o_d[c][:, HK // 2:HK, :],
                                  ocC[:, HK // 2:HK, :])

    nc.compile()
    return nc


def _get_program(key):
    if key not in _cached:
        _cached[key] = _build_program(key)
    return _cached[key]


def _prepare(input, h_mask, v_mask):
    x = np.asarray(input, dtype=np.float32)
    hm = np.asarray(h_mask, dtype=np.int32)
    vm = np.asarray(v_mask, dtype=np.int32)
    assert x.shape == (1, H, W, C), x.shape

    row_ids = _segment_ids(hm[0])
    col_ids = _segment_ids(vm[0])
    nr = int(row_ids[-1]) + 1
    ncs = int(col_ids[-1]) + 1
    NRP = ((nr + 63) // 64) * 64
    NCP = ((ncs + 63) // 64) * 64

    rblocks, rassign = _adapt_blocks(row_ids, 128, nr)
    qblocks, qassign = _adapt_blocks(col_ids, NB, ncs)
    key = (NRP, NCP,
           tuple(rblocks), tuple(tuple(a) for a in rassign),
           tuple(qblocks), tuple(tuple(a) for a in qassign))

    n_r = np.bincount(row_ids, minlength=NRP).astype(np.float64)  # [NRP]
    n_q = np.bincount(col_ids, minlength=NCP).astype(np.float64)  # [NCP]

    # one-hot assignment matrices
    ah = np.zeros((H, NRP), np.float32)
    ah[np.arange(H), row_ids] = 1.0
    aw = np.zeros((W, NCP), np.float32)
    aw[np.arange(W), col_ids] = 1.0

    inv_full = np.zeros((NCP, NRP), np.float64)
    valid = np.outer(n_q > 0, n_r > 0)
    denom = np.outer(n_q, n_r)
    inv_full[valid] = 1.0 / denom[valid]

    # per-adapted-block partition layouts (zero padded to 128 partitions)
    QBn, RBn = len(qblocks), len(rblocks)
    inv_dev = np.zeros((128, QBn, NRP), np.float32)
    awtb_dev = np.zeros((128, QBn, W), np.float32)
    for b, (qo, qs) in enumerate(qblocks):
        inv_dev[0:qs, b, :] = inv_full[qo:qo + qs]
        awtb_dev[0:qs, b, :] = aw.T[qo:qo + qs]
    ahtb_dev = np.zeros((128, RBn, H), np.float32)
    for b, (ro, rs) in enumerate(rblocks):
        ahtb_dev[0:rs, b, :] = ah.T[ro:ro + rs]

    ahn_dev = np.ascontiguousarray(
        ah.reshape(HK, 128, NRP).transpose(1, 0, 2)).astype(NPDT)
    awn_dev = np.ascontiguousarray(
        aw.reshape(WK, 128, NCP).transpose(1, 0, 2)).astype(NPDT)
    inv_dev = np.ascontiguousarray(inv_dev)
    awtb_dev = awtb_dev.astype(NPDT)
    ahtb_dev = ahtb_dev.astype(NPDT)

    # per-core planar input: [CL, 128(p), HK(h0), W] with h = 128*h0 + p
    x64 = x[0].transpose(2, 0, 1)  # [C, H, W]
    in_maps = []
    for core in range(NCORES):
        xc = x64[CL * core:CL * (core + 1)]  # [CL, H, W]
        xdev = np.ascontiguousarray(
            xc.reshape(CL, HK, 128, W).transpose(0, 2, 1, 3)).astype(NPDT)
        in_maps.append({
            "x": xdev,
            "ahn": ahn_dev,
            "awn": awn_dev,
            "invt": inv_dev,
            "awtb": awtb_dev,
            "ahtb": ahtb_dev,
        })
    return in_maps, key


def _assemble(results):
    out = np.empty((1, H, W, C), np.float32)
    for core in range(NCORES):
        o = np.asarray(results[core]["o"]).astype(np.float32)  # [CL,128,HK,W]
        oc = o.transpose(0, 2, 1, 3).reshape(CL, H, W)         # h = 128*m + p
        out[0, :, :, CL * core:CL * (core + 1)] = oc.transpose(1, 2, 0)
    return out


def run(inputs: dict, trace: bool = False, **kwargs):
    """Full pipeline; returns (output, BassKernelResults)."""
    in_maps, key = _prepare(**inputs)
    nc = _get_program(key)
    res = run_bass_kernel_spmd(nc, in_maps, list(range(NCORES)),
                               trace=trace, **kwargs)
    return _assemble(res.results), res


def kernel(**inputs) -> np.ndarray:
    out, _ = run(inputs, trace=False)
    return out


# revision 13
# speedup vs baseline: 4.0350x; 1.0267x over previous
"""Grid pooling (segment mean over rectangular grid cells) on 8 trn2 cores.

Math: row/col masks induce contiguous run-segments along H and W, so every
grid cell is a rectangle and the whole op factorizes per channel as

    out_c = A_h @ diag-scale( A_h^T @ X_c @ A_w ) @ A_w^T

with one-hot segment-assignment matrices A_h [H, NR], A_w [W, NC] built on
host from the tiny masks. Channels (64) are sharded 8-way across cores, so
each core runs 8 independent 768x768 channel planes through 4 matmul stages:

  1. R^T  = X_c^T  @ A_h      (row-segment sums;   lhsT = X chunks)
  2. S^T  = A_w^T  @ R^T      (col-segment sums;   lhsT = A_w chunks)
     S̄^T = S^T * 1/(n_r*n_q) (DVE multiply while copying PSUM->SBUF)
  3. U    = S̄     @ A_w^Tb   (broadcast cols back; lhsT = S̄^T chunks)
  4. OUT  = A_h b  @ U        (broadcast rows back; lhsT = A_h^T chunks)

Data is bf16 on-chip (fp32 PSUM accumulation, fp32 scale factors): the PE
runs 1 cycle/row instead of fp32's 4 and HBM traffic halves.

Because segments are contiguous runs, the segments touched by one 128-row
output block (or one 384-column tile) span a narrow index range (~33 for
random masks). The r/q axes are therefore partitioned into mask-adapted,
possibly overlapping blocks of <=128 segments such that every output tile's
range lives in ONE block, making stages 3 and 4 single-k matmuls. Falls
back to fixed disjoint 128-blocks (multi-k accumulation) if a range is too
wide. The program is cached per blocking structure.
"""

import numpy as np
import ml_dtypes

from concourse import bacc, tile
import concourse.mybir as mybir
from concourse.bass_utils import run_bass_kernel_spmd

H = 768
W = 768
C = 64
NCORES = 8
CL = C // NCORES          # channels per core
HK = H // 128             # 6 H-chunks (contraction / output chunks)
WK = W // 128             # 6 W-chunks
NB = 384                  # free-dim tile for broadcast stages (768 = 2*384)

DT = mybir.dt.bfloat16    # on-chip data dtype for X/R/S/U/out
F32 = mybir.dt.float32
NPDT = ml_dtypes.bfloat16

_cached = {}


def _segment_ids(mask: np.ndarray) -> np.ndarray:
    """mask [L] binary -> segment ids via rising edges (pixel 0 -> seg 0)."""
    m = mask.astype(np.int64)
    prev = np.concatenate([[0], m[:-1]])
    rising = (m == 1) & (prev == 0)
    rising[0] = False
    return np.cumsum(rising.astype(np.int64)).astype(np.int32)


def _adapt_blocks(ids: np.ndarray, tile_len: int, nseg: int):
    """Partition the segment axis into blocks of <=128 ids such that the id
    range of every output tile of `tile_len` positions lies in one block.

    Returns (blocks, assign): blocks = [(start, width), ...] (may overlap by
    a shared boundary segment), assign[t] = [(block_idx, k_first), ...] the
    block(s) tile t accumulates over. Single-element lists on the fast path;
    falls back to fixed disjoint 128-blocks (multi-k) if a tile's range
    exceeds 128 segments.
    """
    L = len(ids)
    ntiles = L // tile_len
    ranges = [(int(ids[t * tile_len]), int(ids[(t + 1) * tile_len - 1]))
              for t in range(ntiles)]
    if all(hi - lo + 1 <= 128 for lo, hi in ranges):
        blocks, assign = [], []
        cur_start, cur_end = 0, -1          # current block [cur_start, cur_end]
        for lo, hi in ranges:
            if hi - cur_start + 1 <= 128 and cur_end >= 0:
                cur_end = max(cur_end, hi)
            else:
                if cur_end >= 0:
                    blocks.append((cur_start, cur_end - cur_start + 1))
                cur_start, cur_end = lo, hi
            assign.append(len(blocks))      # block this tile will land in
        blocks.append((cur_start, cur_end - cur_start + 1))
        return blocks, [[a] for a in assign]
    # fallback: fixed disjoint blocks, tiles accumulate over all their blocks
    blocks = [(off, min(128, nseg - off)) for off in range(0, nseg, 128)]
    assign = []
    for lo, hi in ranges:
        assign.append([b for b, (s, w) in enumerate(blocks)
                       if not (hi < s or lo > s + w - 1)])
    return blocks, assign


def _build_program(key):
    (NRP, NCP, rblocks, rassign, qblocks, qassign) = key
    RBn = len(rblocks)
    QBn = len(qblocks)

    nc = bacc.Bacc("TRN2", target_bir_lowering=False, debug=False,
                   num_devices=NCORES)

    x_d = nc.dram_tensor("x", [CL, 128, HK, W], DT, kind="ExternalInput")
    ahn_d = nc.dram_tensor("ahn", [128, HK, NRP], DT, kind="ExternalInput")
    awn_d = nc.dram_tensor("awn", [128, WK, NCP], DT, kind="ExternalInput")
    inv_d = nc.dram_tensor("invt", [128, QBn, NRP], F32, kind="ExternalInput")
    awtb_d = nc.dram_tensor("awtb", [128, QBn, W], DT, kind="ExternalInput")
    ahtb_d = nc.dram_tensor("ahtb", [128, RBn, H], DT, kind="ExternalInput")
    o_d = nc.dram_tensor("o", [CL, 128, HK, W], DT, kind="ExternalOutput")

    with tile.TileContext(nc) as tc:
        with (
            tc.tile_pool(name="const", bufs=1) as constp,
            tc.tile_pool(name="xp", bufs=3) as xp,
            tc.tile_pool(name="rp", bufs=2) as rp,
            tc.tile_pool(name="sp", bufs=2) as sp,
            tc.tile_pool(name="up", bufs=2) as up,
            tc.tile_pool(name="op", bufs=2) as op_,
            # 8 PSUM banks total: psr 2 + pss 1 + psu 2 + pso 3
            tc.tile_pool(name="psr", bufs=2, space="PSUM") as psr,
            tc.tile_pool(name="pss", bufs=1, space="PSUM") as pss,
            tc.tile_pool(name="psu", bufs=2, space="PSUM") as psu,
            tc.tile_pool(name="pso", bufs=3, space="PSUM") as pso,
        ):
            # two stage-1 results fit one 2KB PSUM bank when NRP <= 256
            pair1 = 2 * NRP * 4 <= 2048
            # first channel's input before the big broadcast constants so the
            # PE can start as early as possible (ahn is all stage 1 needs);
            # halves land on different DMA queues and overlap
            xc0 = xp.tile([128, HK, W], DT)
            nc.sync.dma_start(xc0[:, 0:HK // 2, :], x_d[0][:, 0:HK // 2, :])
            ahn = constp.tile([128, HK, NRP], DT)
            nc.sync.dma_start(ahn[:], ahn_d[:])
            nc.sync.dma_start(xc0[:, HK // 2:HK, :], x_d[0][:, HK // 2:HK, :])
            awn = constp.tile([128, WK, NCP], DT)
            nc.sync.dma_start(awn[:], awn_d[:])
            invt = constp.tile([128, QBn, NRP], F32)
            nc.sync.dma_start(invt[:], inv_d[:])
            awtb = constp.tile([128, QBn, W], DT)
            nc.sync.dma_start(awtb[:], awtb_d[:])
            ahtb = constp.tile([128, RBn, H], DT)
            nc.sync.dma_start(ahtb[:], ahtb_d[:])

            for c in range(CL):
                if c == 0:
                    xc = xc0
                else:
                    xc = xp.tile([128, HK, W], DT)
                    nc.sync.dma_start(xc[:], x_d[c])

                # stage 1: R^T[j, r] per W-chunk m (contract H in 6 chunks);
                # two m per PSUM bank, one paired copy out
                rc = rp.tile([128, WK, NRP], DT)
                if pair1:
                    for mp in range(WK // 2):
                        pr = psr.tile([128, 2, NRP], F32)
                        for half in range(2):
                            m = 2 * mp + half
                            for k in range(HK):
                                nc.tensor.matmul(
                                    pr[:, half, :],
                                    xc[:, k, 128 * m:128 * m + 128],
                                    ahn[:, k, :],
                                    start=(k == 0), stop=(k == HK - 1),
                                )
                        nc.scalar.copy(rc[:, 2 * mp:2 * mp + 2, :], pr[:])
                else:
                    for m in range(WK):
                        pr = psr.tile([128, NRP], F32)
                        for k in range(HK):
                            nc.tensor.matmul(
                                pr[:],
                                xc[:, k, 128 * m:128 * m + 128],
                                ahn[:, k, :],
                                start=(k == 0), stop=(k == HK - 1),
                            )
                        nc.scalar.copy(rc[:, m, :], pr[:])

                # stage 2: S^T[q, r] (contract W in 6 chunks), scale by
                # 1/(n_r*n_q) while copying out of PSUM
                sc = sp.tile([128, QBn, NRP], DT)
                for b, (qo, qs) in enumerate(qblocks):
                    ps = pss.tile([128, NRP], F32)
                    for k in range(WK):
                        nc.tensor.matmul(
                            ps[0:qs, :],
                            awn[:, k, qo:qo + qs],
                            rc[:, k, :],
                            start=(k == 0), stop=(k == WK - 1),
                        )
                    nc.vector.tensor_mul(sc[0:qs, b, :], ps[0:qs, :],
                                         invt[0:qs, b, :])

                # stage 3: U[r, j] = S̄[r, col_ids(j)] per adapted r-block
                uc = up.tile([128, RBn, W], DT)
                for ri, (ro, rs) in enumerate(rblocks):
                    for n in range(W // NB):
                        pu = psu.tile([128, NB], F32)
                        ks = qassign[n]
                        for j, b in enumerate(ks):
                            qo, qs = qblocks[b]
                            nc.tensor.matmul(
                                pu[0:rs, :],
                                sc[0:qs, b, ro:ro + rs],
                                awtb[0:qs, b, NB * n:NB * n + NB],
                                start=(j == 0), stop=(j == len(ks) - 1),
                            )
                        nc.scalar.copy(uc[0:rs, ri, NB * n:NB * n + NB],
                                       pu[0:rs, :])

                # stage 4: OUT[i, j] = U[row_ids(i), j] into the channel tile;
                # casts split 3:1 vector:scalar, output DMA'd in two halves
                ocC = op_.tile([128, HK, W], DT)
                for m in range(HK):
                    for n in range(W // NB):
                        po = pso.tile([128, NB], F32)
                        ks = rassign[m]
                        for j, b in enumerate(ks):
                            ro, rs = rblocks[b]
                            nc.tensor.matmul(
                                po[:],
                                ahtb[0:rs, b, 128 * m:128 * m + 128],
                                uc[0:rs, b, NB * n:NB * n + NB],
                                start=(j == 0), stop=(j == len(ks) - 1),
                            )
                        dst = ocC[:, m, NB * n:NB * n + NB]
                        if (2 * m + n) % 4 == 3:
                            nc.scalar.copy(dst, po[:])
                        else:
                            nc.vector.tensor_copy(dst, po[:])
                    if m % 2 == 1 and m < HK - 1:
                        nc.sync.dma_start(o_d[c][:, m - 1:m + 1, :],
                                          ocC[:, m - 1:m + 1, :])
                nc.sync.dma_start(o_d[c][:, HK - 2:HK, :],
                                  ocC[:, HK - 2:HK, :])

    nc.compile()
    return nc


def _get_program(key):
    if key not in _cached:
        _cached[key] = _build_program(key)
    return _cached[key]


def _prepare(input, h_mask, v_mask):
    x = np.asarray(input, dtype=np.float32)
    hm = np.asarray(h_mask, dtype=np.int32)
    vm = np.asarray(v_mask, dtype=np.int32)
    assert x.shape == (1, H, W, C), x.shape

    row_ids = _segment_ids(hm[0])
    col_ids = _segment_ids(vm[0])
    nr = int(row_ids[-1]) + 1
    ncs = int(col_ids[-1]) + 1
    NRP = ((nr + 63) // 64) * 64
    NCP = ((ncs + 63) // 64) * 64

    rblocks, rassign = _adapt_blocks(row_ids, 128, nr)
    qblocks, qassign = _adapt_blocks(col_ids, NB, ncs)
    key = (NRP, NCP,
           tuple(rblocks), tuple(tuple(a) for a in rassign),
           tuple(qblocks), tuple(tuple(a) for a in qassign))

    n_r = np.bincount(row_ids, minlength=NRP).astype(np.float64)  # [NRP]
    n_q = np.bincount(col_ids, minlength=NCP).astype(np.float64)  # [NCP]

    # one-hot assignment matrices
    ah = np.zeros((H, NRP), np.float32)
    ah[np.arange(H), row_ids] = 1.0
    aw = np.zeros((W, NCP), np.float32)
    aw[np.arange(W), col_ids] = 1.0

    inv_full = np.zeros((NCP, NRP), np.float64)
    valid = np.outer(n_q > 0, n_r > 0)
    denom = np.outer(n_q, n_r)
    inv_full[valid] = 1.0 / denom[valid]

    # per-adapted-block partition layouts (zero padded to 128 partitions)
    QBn, RBn = len(qblocks), len(rblocks)
    inv_dev = np.zeros((128, QBn, NRP), np.float32)
    awtb_dev = np.zeros((128, QBn, W), np.float32)
    for b, (qo, qs) in enumerate(qblocks):
        inv_dev[0:qs, b, :] = inv_full[qo:qo + qs]
        awtb_dev[0:qs, b, :] = aw.T[qo:qo + qs]
    ahtb_dev = np.zeros((128, RBn, H), np.float32)
    for b, (ro, rs) in enumerate(rblocks):
        ahtb_dev[0:rs, b, :] = ah.T[ro:ro + rs]

    ahn_dev = np.ascontiguousarray(
        ah.reshape(HK, 128, NRP).transpose(1, 0, 2)).astype(NPDT)
    awn_dev = np.ascontiguousarray(
        aw.reshape(WK, 128, NCP).transpose(1, 0, 2)).astype(NPDT)
    inv_dev = np.ascontiguousarray(inv_dev)
    awtb_dev = awtb_dev.astype(NPDT)
    ahtb_dev = ahtb_dev.astype(NPDT)

    # per-core planar input: [CL, 128(p), HK(h0), W] with h = 128*h0 + p
    x64 = x[0].transpose(2, 0, 1)  # [C, H, W]
    in_maps = []
    for core in range(NCORES):
        xc = x64[CL * core:CL * (core + 1)]  # [CL, H, W]
        xdev = np.ascontiguousarray(
            xc.reshape(CL, HK, 128, W).transpose(0, 2, 1, 3)).astype(NPDT)
        in_maps.append({
            "x": xdev,
            "ahn": ahn_dev,
            "awn": awn_dev,
            "invt": inv_dev,
            "awtb": awtb_dev,
            "ahtb": ahtb_dev,
        })
    return in_maps, key


def _assemble(results):
    out = np.empty((1, H, W, C), np.float32)
    for core in range(NCORES):
        o = np.asarray(results[core]["o"]).astype(np.float32)  # [CL,128,HK,W]
        oc = o.transpose(0, 2, 1, 3).reshape(CL, H, W)         # h = 128*m + p
        out[0, :, :, CL * core:CL * (core + 1)] = oc.transpose(1, 2, 0)
    return out


def run(inputs: dict, trace: bool = False, **kwargs):
    """Full pipeline; returns (output, BassKernelResults)."""
    in_maps, key = _prepare(**inputs)
    nc = _get_program(key)
    res = run_bass_kernel_spmd(nc, in_maps, list(range(NCORES)),
                               trace=trace, **kwargs)
    return _assemble(res.results), res


def kernel(**inputs) -> np.ndarray:
    out, _ = run(inputs, trace=False)
    return out
